# revision 12
# baseline (speedup 1.0000x reference)
"""MinimalGRU (2-layer) Trainium2 Bass kernel, data-parallel over batch on 8 cores.

Full inputs in, full output out. Per core: 4 sequences.

The whole recurrence runs in a TRANSPOSED layout: h^T lives as a [128, 32]
SBUF tile (partition p, col 4k+b  <->  h[b, 128k+p]).  Gates are computed
transposed -- for each 128-row gate chunk the weight tile W_hh^T[128k, 128c]
is the PE stationary operand and the tiny h^T chunk [128, 4] is the moving
operand, so each matmul streams only 4 columns.  Per-step pre-activations
(x@W_ih^T + biases, precomputed by GEMM phases) are injected into the same
PSUM accumulation group through an identity-stationary matmul, and biases are
folded in by the GEMM epilogues (ACT Identity with per-partition bias).
No transpose-rebuild matmuls are needed anywhere: the elementwise GRU update
runs directly on the transposed tiles and its bf16 output is simultaneously
next step's h^T, the layer-1 GEMM moving operand (layer 0), and the DMA
output staging (layer 1).  Layer 1 lags layer 0 by LAG steps; the layer-1
input GEMM is spread one gate-chunk per step to keep PE bursts short.
"""

import os
import numpy as np
import ml_dtypes

import concourse.bass as bass  # noqa: F401
import concourse.mybir as mybir
from concourse import bacc
from concourse.tile import TileContext
from concourse.bass_utils import run_bass_kernel_spmd

BF16 = ml_dtypes.bfloat16
F32 = np.float32

H = 1024
DX = 512
G = 2 * H          # 2048 gate rows (transposed layout)
B = 32
NCORES = 8
BL = B // NCORES   # 4 sequences per core
T = int(os.environ.get("GRU_T", "512"))

KC = H // 128      # 8 h chunks
GC = G // 128      # 16 gate chunks (0..7 = update gate, 8..15 = candidate)
W = 32             # window (steps) for pre fetch / store / L1 GEMM
NW = T // W
LAG = 52           # layer-1 step lag behind layer 0 (>= W + 16 + margin)

_CACHE: dict = {}


class _LS:
    pass


def _build():
    fp32 = mybir.dt.float32
    bf16 = mybir.dt.bfloat16
    add = mybir.AluOpType.add
    nc = bacc.Bacc("TRN2", target_bir_lowering=False, debug=False,
                   num_devices=NCORES)

    xt = nc.dram_tensor("xt", [DX, BL * T], bf16, kind="ExternalInput")
    wht0 = nc.dram_tensor("wht0", [H, G], bf16, kind="ExternalInput")
    wht1 = nc.dram_tensor("wht1", [H, G], bf16, kind="ExternalInput")
    wih0t = nc.dram_tensor("wih0t", [DX, G], bf16, kind="ExternalInput")
    wih1t = nc.dram_tensor("wih1t", [H, G], bf16, kind="ExternalInput")
    b0c = nc.dram_tensor("b0c", [128, GC], fp32, kind="ExternalInput")
    b1c = nc.dram_tensor("b1c", [128, GC], fp32, kind="ExternalInput")
    h0t = nc.dram_tensor("h0t", [128, 32], bf16, kind="ExternalInput")
    h1t = nc.dram_tensor("h1t", [128, 32], bf16, kind="ExternalInput")
    idm = nc.dram_tensor("idm", [128, 128], bf16, kind="ExternalInput")
    out_d = nc.dram_tensor("out", [128, T, 32], bf16, kind="ExternalOutput")

    pre0_d = nc.dram_tensor("pre0_d", [GC, 128, T, BL], bf16, kind="Internal")

    with TileContext(nc) as tc:
        with tc.tile_pool(name="wconst", bufs=1) as wconst:
            w0_t = [wconst.tile([128, G], bf16, tag=f"w0_{k}", name=f"w0_{k}")
                    for k in range(KC)]
            w1_t = [wconst.tile([128, G], bf16, tag=f"w1_{k}", name=f"w1_{k}")
                    for k in range(KC)]
            wih1_t = [wconst.tile([128, G], bf16, tag=f"wih1_{k}",
                                  name=f"wih1_{k}") for k in range(KC)]
            for k in range(KC):
                nc.sync.dma_start(w0_t[k][:, :], wht0[128 * k: 128 * k + 128, :])
                nc.sync.dma_start(w1_t[k][:, :], wht1[128 * k: 128 * k + 128, :])
                nc.sync.dma_start(wih1_t[k][:, :],
                                  wih1t[128 * k: 128 * k + 128, :])
            b0_t = wconst.tile([128, GC], fp32, tag="b0", name="b0")
            b1_t = wconst.tile([128, GC], fp32, tag="b1", name="b1")
            id_t = wconst.tile([128, 128], bf16, tag="idm", name="idm")
            h0t_t = wconst.tile([128, 32], bf16, tag="h0t", name="h0t")
            h1t_t = wconst.tile([128, 32], bf16, tag="h1t", name="h1t")
            for dst, src in ((b0_t, b0c), (b1_t, b1c), (id_t, idm),
                             (h0t_t, h0t), (h1t_t, h1t)):
                nc.sync.dma_start(dst[:, :], src[:, :])

            # ---- Phase B: layer-0 input GEMM -> pre0_d (bias folded in)
            with (
                tc.tile_pool(name="p1x", bufs=1) as p1x,
                tc.tile_pool(name="p1ps", bufs=2, space="PSUM") as p1ps,
                tc.tile_pool(name="p1o", bufs=3) as p1o,
            ):
                xt_t = [p1x.tile([128, BL * T], bf16, tag=f"xt{k}",
                                 name=f"xtt{k}") for k in range(DX // 128)]
                wih0_t = [p1x.tile([128, G], bf16, tag=f"wih0_{k}",
                                   name=f"wih0_{k}") for k in range(DX // 128)]
                for k in range(DX // 128):
                    nc.sync.dma_start(xt_t[k][:, :],
                                      xt[128 * k: 128 * k + 128, :])
                    nc.sync.dma_start(wih0_t[k][:, :],
                                      wih0t[128 * k: 128 * k + 128, :])
                CH = min(512, BL * T)   # (t,b)-column chunk per matmul
                NT = BL * T // CH
                TCH = CH // BL
                for c in range(GC):
                    for n in range(NT):
                        pp = p1ps.tile([128, CH], fp32, tag="pp", name="pp")
                        for k in range(DX // 128):
                            nc.tensor.matmul(
                                pp[:, :],
                                wih0_t[k][:, 128 * c: 128 * c + 128],
                                xt_t[k][:, CH * n: CH * n + CH],
                                start=(k == 0), stop=(k == DX // 128 - 1),
                            )
                        po = p1o.tile([128, CH], bf16, tag="po", name="po")
                        nc.scalar.activation(
                            po[:, :], pp[:, :],
                            mybir.ActivationFunctionType.Identity,
                            bias=b0_t[:, c:c + 1])
                        nc.sync.dma_start(
                            pre0_d[c, :, TCH * n: TCH * n + TCH, :],
                            po.rearrange("p (t b) -> p t b", b=BL))

            tc.strict_bb_all_engine_barrier()

            # ---- Phase C: the two recurrent layers, interleaved
            with (
                tc.tile_pool(name="p0w", bufs=3) as p0w_pool,
                tc.tile_pool(name="p1w", bufs=3) as p1w_pool,
                tc.tile_pool(name="st0", bufs=2) as st0_pool,
                tc.tile_pool(name="st1", bufs=2) as st1_pool,
                tc.tile_pool(name="tmp", bufs=4) as tmp_pool,
                tc.tile_pool(name="g0ps", bufs=2, space="PSUM") as g0ps,
                tc.tile_pool(name="g1ps", bufs=2, space="PSUM") as g1ps,
                tc.tile_pool(name="gps", bufs=2, space="PSUM") as g_ps,
            ):
                pre0_tiles: dict = {}
                pre1_tiles: dict = {}
                st0_tiles: dict = {}

                def fetch_pre0(w):
                    t_ = p0w_pool.tile([128, GC, W, BL], bf16, tag="p0w",
                                       name="p0w")
                    for c in range(GC):
                        nc.sync.dma_start(
                            t_[:, c, :, :],
                            pre0_d[c, :, W * w: W * w + W, :])
                    pre0_tiles[w] = t_

                gemm_pg = [None]

                def emit_gemm_half(w, c, half):
                    src = st0_tiles[w]
                    if half == 0:
                        if c == 0:
                            pre1_tiles[w] = p1w_pool.tile(
                                [128, GC, W, BL], bf16, tag="p1w", name="p1w")
                        gemm_pg[0] = g_ps.tile([128, W, BL], fp32, tag="pg",
                                               name="pg")
                    pg = gemm_pg[0]
                    for k in range(4 * half, 4 * half + 4):
                        nc.tensor.matmul(
                            pg[:, :, :],
                            wih1_t[k][:, 128 * c: 128 * c + 128],
                            src[:, :, 4 * k: 4 * k + 4],
                            start=(k == 0), stop=(k == KC - 1),
                        )
                    if half == 1:
                        nc.scalar.activation(
                            pre1_tiles[w][:, c, :, :], pg[:, :, :],
                            mybir.ActivationFunctionType.Identity,
                            bias=b1_t[:, c:c + 1])

                L0 = _LS()
                L0.idx, L0.w_t = 0, w0_t
                L0.gps = g0ps
                L0.st_pool = st0_pool
                L0.h_prev = h0t_t[:, :]
                L1 = _LS()
                L1.idx, L1.w_t = 1, w1_t
                L1.gps = g1ps
                L1.st_pool = st1_pool
                L1.h_prev = h1t_t[:, :]

                def emit_step(L, s):
                    li = L.idx
                    w, slot = divmod(s, W)
                    if slot == 0:
                        L.stage = L.st_pool.tile([128, W, 32], bf16,
                                                 tag=f"st{li}", name=f"st{li}")
                        if li == 0:
                            st0_tiles[w] = L.stage
                            st0_tiles.pop(w - 2, None)
                            if w + 1 < NW:
                                fetch_pre0(w + 1)
                            pre0_tiles.pop(w - 1, None)
                    pre_t = pre0_tiles[w] if li == 0 else pre1_tiles[w]
                    gp = L.gps.tile([128, 64], fp32, tag=f"g{li}",
                                    name=f"g{li}")
                    for half in (0, 1):
                        nc.tensor.matmul(
                            gp[:, 32 * half: 32 * half + 32], id_t[:, :],
                            pre_t[:, 8 * half: 8 * half + 8, slot, :],
                            start=True, stop=False, skip_group_check=True)
                        for c8 in range(8):
                            c = 8 * half + c8
                            out_ap = gp[:, 32 * half + 4 * c8:
                                        32 * half + 4 * c8 + 4]
                            for k in range(KC):
                                nc.tensor.matmul(
                                    out_ap,
                                    L.w_t[k][:, 128 * c: 128 * c + 128],
                                    L.h_prev[:, 4 * k: 4 * k + 4],
                                    start=False, stop=(k == KC - 1),
                                    skip_group_check=True)
                    up = tmp_pool.tile([128, 32], bf16, tag=f"up{li}",
                                       name=f"up{li}")
                    nc.scalar.activation(up[:, :], gp[:, 0:32],
                                         mybir.ActivationFunctionType.Sigmoid,
                                         scale=-1.0)
                    d = tmp_pool.tile([128, 32], bf16, tag=f"d{li}",
                                      name=f"d{li}")
                    nc.vector.scalar_tensor_tensor(
                        d[:, :], gp[:, 32:64], 0.0, L.h_prev,
                        mybir.AluOpType.max, mybir.AluOpType.subtract)
                    nc.vector.tensor_tensor(d[:, :], d[:, :], up[:, :],
                                            mybir.AluOpType.mult)
                    hn = L.stage[:, slot, :]
                    nc.vector.tensor_tensor(hn, L.h_prev, d[:, :], add)
                    L.h_prev = L.stage[:, slot, :]
                    if li == 1 and slot == W - 1:
                        nc.sync.dma_start(out_d[:, W * w: W * w + W, :],
                                          L.stage[:, :, :])

                fetch_pre0(0)
                for tt in range(T + LAG):
                    do_gemm = tt >= W and tt % W < GC and (tt // W - 1) < NW
                    if tt < T:
                        emit_step(L0, tt)
                    if do_gemm:
                        emit_gemm_half(tt // W - 1, tt % W, 0)
                    if LAG <= tt < T + LAG:
                        emit_step(L1, tt - LAG)
                    if do_gemm:
                        emit_gemm_half(tt // W - 1, tt % W, 1)

    nc.compile()
    return nc


def _prep_core(inputs, c, shared):
    x = inputs["x"][BL * c: BL * c + BL, :T]          # [4, T, DX]
    xt = np.ascontiguousarray(
        x.transpose(2, 1, 0).reshape(DX, T * BL)).astype(BF16)

    def htr(hv):                                      # [4, H] -> [128, 32]
        o = np.zeros((128, 32), F32)
        for k in range(KC):
            o[:, 4 * k: 4 * k + 4] = hv[:, 128 * k: 128 * k + 128].T
        return o

    h0 = inputs["hx"][0, BL * c: BL * c + BL]
    h1 = inputs["hx"][1, BL * c: BL * c + BL]
    return {
        "xt": xt,
        "h0t": htr(h0).astype(BF16), "h1t": htr(h1).astype(BF16),
        **shared,
    }


def get_nc():
    nc = _CACHE.get("nc")
    if nc is None:
        nc = _build()
        _CACHE["nc"] = nc
    return nc


def make_in_maps(inputs):
    inputs = {k: np.asarray(v) for k, v in inputs.items()}
    shared = {
        "wht0": np.ascontiguousarray(inputs["w_hh_l0"].T).astype(BF16),
        "wht1": np.ascontiguousarray(inputs["w_hh_l1"].T).astype(BF16),
        "wih0t": np.ascontiguousarray(inputs["w_ih_l0"].T).astype(BF16),
        "wih1t": np.ascontiguousarray(inputs["w_ih_l1"].T).astype(BF16),
        "b0c": np.ascontiguousarray(
            (inputs["b_ih_l0"] + inputs["b_hh_l0"]).astype(F32)
            .reshape(GC, 128).T),
        "b1c": np.ascontiguousarray(
            (inputs["b_ih_l1"] + inputs["b_hh_l1"]).astype(F32)
            .reshape(GC, 128).T),
        "idm": np.eye(128, dtype=BF16),
    }
    return [_prep_core(inputs, c, shared) for c in range(NCORES)]


def kernel(**inputs) -> np.ndarray:
    nc = get_nc()
    in_maps = make_in_maps(inputs)
    try:
        res = run_bass_kernel_spmd(nc, in_maps, core_ids=list(range(NCORES)))
    except Exception:
        # a previously wedged device often recovers on the next attempt
        import time
        time.sleep(2.0)
        res = run_bass_kernel_spmd(nc, in_maps, core_ids=list(range(NCORES)))
    outs = []
    for c in range(NCORES):
        o = np.asarray(res.results[c]["out"], F32)      # [128, T, 32]
        o = o.reshape(128, T, KC, BL).transpose(3, 1, 2, 0)
        outs.append(np.ascontiguousarray(o).reshape(BL, T, H))
    return np.concatenate(outs, axis=0).astype(F32)


# revision 30
# speedup vs baseline: 1.0284x; 1.0284x over previous
"""MinimalGRU (2-layer) Trainium2 Bass kernel, data-parallel over batch on 8 cores.

Full inputs in, full output out. Per core: 4 sequences.

The whole recurrence runs in a TRANSPOSED layout: h^T lives as a [128, 32]
SBUF tile (partition p, col 4k+b  <->  h[b, 128k+p]).  Gates are computed
transposed -- for each 128-row gate chunk the weight tile W_hh^T[128k, 128c]
is the PE stationary operand and the tiny h^T chunk [128, 4] is the moving
operand, so each matmul streams only 4 columns.  Per-step pre-activations
(x@W_ih^T + biases, precomputed by GEMM phases) are injected into the same
PSUM accumulation group through an identity-stationary matmul, and biases are
folded in by the GEMM epilogues (ACT Identity with per-partition bias).
No transpose-rebuild matmuls are needed anywhere: the elementwise GRU update
runs directly on the transposed tiles and its bf16 output is simultaneously
next step's h^T, the layer-1 GEMM moving operand (layer 0), and the DMA
output staging (layer 1).  Layer 1 lags layer 0 by LAG steps; the layer-1
input GEMM is spread one gate-chunk per step to keep PE bursts short.
"""

import os
import numpy as np
import ml_dtypes

import concourse.bass as bass  # noqa: F401
import concourse.mybir as mybir
from concourse import bacc
from concourse.tile import TileContext
from concourse.bass_utils import run_bass_kernel_spmd

BF16 = ml_dtypes.bfloat16
F32 = np.float32

H = 1024
DX = 512
G = 2 * H          # 2048 gate rows (transposed layout)
B = 32
NCORES = 8
BL = B // NCORES   # 4 sequences per core
T = int(os.environ.get("GRU_T", "512"))

KC = H // 128      # 8 h chunks
GC = G // 128      # 16 gate chunks (0..7 = update gate, 8..15 = candidate)
W = 32             # window (steps) for pre fetch / store / L1 GEMM
NW = T // W
LAG = 52           # layer-1 step lag behind layer 0 (>= W + 16 + margin)

_CACHE: dict = {}


class _LS:
    pass


def _build():
    fp32 = mybir.dt.float32
    bf16 = mybir.dt.bfloat16
    add = mybir.AluOpType.add
    nc = bacc.Bacc("TRN2", target_bir_lowering=False, debug=False,
                   num_devices=NCORES)

    xt = nc.dram_tensor("xt", [DX, BL * T], bf16, kind="ExternalInput")
    wht0 = nc.dram_tensor("wht0", [H, G], bf16, kind="ExternalInput")
    wht1 = nc.dram_tensor("wht1", [H, G], bf16, kind="ExternalInput")
    wih0t = nc.dram_tensor("wih0t", [DX, G], bf16, kind="ExternalInput")
    wih1t = nc.dram_tensor("wih1t", [H, G], bf16, kind="ExternalInput")
    b0c = nc.dram_tensor("b0c", [128, GC], fp32, kind="ExternalInput")
    b1c = nc.dram_tensor("b1c", [128, GC], fp32, kind="ExternalInput")
    h0t = nc.dram_tensor("h0t", [128, 32], bf16, kind="ExternalInput")
    h1t = nc.dram_tensor("h1t", [128, 32], bf16, kind="ExternalInput")
    idm = nc.dram_tensor("idm", [128, 128], bf16, kind="ExternalInput")
    out_d = nc.dram_tensor("out", [128, T, 32], bf16, kind="ExternalOutput")

    pre0_d = nc.dram_tensor("pre0_d", [GC, 128, T, BL], bf16, kind="Internal")

    with TileContext(nc) as tc:
        with tc.tile_pool(name="wconst", bufs=1) as wconst:
            w0_t = [wconst.tile([128, G], bf16, tag=f"w0_{k}", name=f"w0_{k}")
                    for k in range(KC)]
            w1_t = [wconst.tile([128, G], bf16, tag=f"w1_{k}", name=f"w1_{k}")
                    for k in range(KC)]
            wih1_t = [wconst.tile([128, G], bf16, tag=f"wih1_{k}",
                                  name=f"wih1_{k}") for k in range(KC)]
            # recurrence weights on the ACT DMA queue so the SP queue serves
            # the phase-B operands (xt, wih0) first; wih1 on SP after them
            for k in range(KC):
                nc.scalar.dma_start(w0_t[k][:, :],
                                    wht0[128 * k: 128 * k + 128, :])
                nc.scalar.dma_start(w1_t[k][:, :],
                                    wht1[128 * k: 128 * k + 128, :])
            b0_t = wconst.tile([128, GC], fp32, tag="b0", name="b0")
            b1_t = wconst.tile([128, GC], fp32, tag="b1", name="b1")
            id_t = wconst.tile([128, 128], bf16, tag="idm", name="idm")
            h0t_t = wconst.tile([128, 32], bf16, tag="h0t", name="h0t")
            h1t_t = wconst.tile([128, 32], bf16, tag="h1t", name="h1t")
            for dst, src in ((b0_t, b0c), (b1_t, b1c), (id_t, idm),
                             (h0t_t, h0t), (h1t_t, h1t)):
                nc.scalar.dma_start(dst[:, :], src[:, :])

            # ---- Phase B: layer-0 input GEMM -> pre0_d (bias folded in)
            with (
                tc.tile_pool(name="p1x", bufs=1) as p1x,
                tc.tile_pool(name="p1ps", bufs=2, space="PSUM") as p1ps,
                tc.tile_pool(name="p1o", bufs=3) as p1o,
            ):
                xt_t = [p1x.tile([128, BL * T], bf16, tag=f"xt{k}",
                                 name=f"xtt{k}") for k in range(DX // 128)]
                wih0_t = [p1x.tile([128, G], bf16, tag=f"wih0_{k}",
                                   name=f"wih0_{k}") for k in range(DX // 128)]
                for k in range(DX // 128):
                    nc.sync.dma_start(xt_t[k][:, :],
                                      xt[128 * k: 128 * k + 128, :])
                    nc.sync.dma_start(wih0_t[k][:, :],
                                      wih0t[128 * k: 128 * k + 128, :])
                for k in range(KC):
                    nc.sync.dma_start(wih1_t[k][:, :],
                                      wih1t[128 * k: 128 * k + 128, :])
                CH = min(512, BL * T)   # (t,b)-column chunk per matmul
                NT = BL * T // CH
                TCH = CH // BL
                for c in range(GC):
                    for n in range(NT):
                        pp = p1ps.tile([128, CH], fp32, tag="pp", name="pp")
                        for k in range(DX // 128):
                            nc.tensor.matmul(
                                pp[:, :],
                                wih0_t[k][:, 128 * c: 128 * c + 128],
                                xt_t[k][:, CH * n: CH * n + CH],
                                start=(k == 0), stop=(k == DX // 128 - 1),
                            )
                        po = p1o.tile([128, CH], bf16, tag="po", name="po")
                        nc.scalar.activation(
                            po[:, :], pp[:, :],
                            mybir.ActivationFunctionType.Identity,
                            bias=b0_t[:, c:c + 1])
                        nc.sync.dma_start(
                            pre0_d[c, :, TCH * n: TCH * n + TCH, :],
                            po.rearrange("p (t b) -> p t b", b=BL))

            tc.strict_bb_all_engine_barrier()

            # ---- Phase C: the two recurrent layers, interleaved
            with (
                tc.tile_pool(name="p0w", bufs=3) as p0w_pool,
                tc.tile_pool(name="p1w", bufs=3) as p1w_pool,
                tc.tile_pool(name="st0", bufs=2) as st0_pool,
                tc.tile_pool(name="st1", bufs=2) as st1_pool,
                tc.tile_pool(name="tmp", bufs=4) as tmp_pool,
                tc.tile_pool(name="g0ps", bufs=2, space="PSUM") as g0ps,
                tc.tile_pool(name="g1ps", bufs=2, space="PSUM") as g1ps,
                tc.tile_pool(name="gps", bufs=2, space="PSUM") as g_ps,
            ):
                pre0_tiles: dict = {}
                pre1_tiles: dict = {}
                st0_tiles: dict = {}

                def fetch_pre0(w):
                    t_ = p0w_pool.tile([128, GC, W, BL], bf16, tag="p0w",
                                       name="p0w")
                    for c in range(GC):
                        nc.sync.dma_start(
                            t_[:, c, :, :],
                            pre0_d[c, :, W * w: W * w + W, :])
                    pre0_tiles[w] = t_

                gemm_pg = [None]

                def emit_gemm_half(w, c, half):
                    src = st0_tiles[w]
                    if half == 0:
                        if c == 0:
                            pre1_tiles[w] = p1w_pool.tile(
                                [128, GC, W, BL], bf16, tag="p1w", name="p1w")
                        gemm_pg[0] = g_ps.tile([128, W, BL], fp32, tag="pg",
                                               name="pg")
                    pg = gemm_pg[0]
                    for k in range(4 * half, 4 * half + 4):
                        nc.tensor.matmul(
                            pg[:, :, :],
                            wih1_t[k][:, 128 * c: 128 * c + 128],
                            src[:, :, 4 * k: 4 * k + 4],
                            start=(k == 0), stop=(k == KC - 1),
                        )
                    if half == 1:
                        nc.scalar.activation(
                            pre1_tiles[w][:, c, :, :], pg[:, :, :],
                            mybir.ActivationFunctionType.Identity,
                            bias=b1_t[:, c:c + 1])

                L0 = _LS()
                L0.idx, L0.w_t = 0, w0_t
                L0.gps = g0ps
                L0.st_pool = st0_pool
                L0.h_prev = h0t_t[:, :]
                L1 = _LS()
                L1.idx, L1.w_t = 1, w1_t
                L1.gps = g1ps
                L1.st_pool = st1_pool
                L1.h_prev = h1t_t[:, :]

                def emit_step(L, s):
                    li = L.idx
                    w, slot = divmod(s, W)
                    if slot == 0:
                        L.stage = L.st_pool.tile([128, W, 32], bf16,
                                                 tag=f"st{li}", name=f"st{li}")
                        if li == 0:
                            st0_tiles[w] = L.stage
                            st0_tiles.pop(w - 2, None)
                            if w + 1 < NW:
                                fetch_pre0(w + 1)
                            pre0_tiles.pop(w - 1, None)
                    pre_t = pre0_tiles[w] if li == 0 else pre1_tiles[w]
                    gp = L.gps.tile([128, 64], fp32, tag=f"g{li}",
                                    name=f"g{li}")
                    for half in (0, 1):
                        nc.tensor.matmul(
                            gp[:, 32 * half: 32 * half + 32], id_t[:, :],
                            pre_t[:, 8 * half: 8 * half + 8, slot, :],
                            start=True, stop=False, skip_group_check=True)
                        for c8 in range(8):
                            c = 8 * half + c8
                            out_ap = gp[:, 32 * half + 4 * c8:
                                        32 * half + 4 * c8 + 4]
                            for k in range(KC):
                                nc.tensor.matmul(
                                    out_ap,
                                    L.w_t[k][:, 128 * c: 128 * c + 128],
                                    L.h_prev[:, 4 * k: 4 * k + 4],
                                    start=False, stop=(k == KC - 1),
                                    skip_group_check=True)
                    up = tmp_pool.tile([128, 32], bf16, tag=f"up{li}",
                                       name=f"up{li}")
                    nc.scalar.activation(up[:, :], gp[:, 0:32],
                                         mybir.ActivationFunctionType.Sigmoid,
                                         scale=-1.0)
                    d = tmp_pool.tile([128, 32], bf16, tag=f"d{li}",
                                      name=f"d{li}")
                    nc.vector.scalar_tensor_tensor(
                        d[:, :], gp[:, 32:64], 0.0, L.h_prev,
                        mybir.AluOpType.max, mybir.AluOpType.subtract)
                    nc.vector.tensor_tensor(d[:, :], d[:, :], up[:, :],
                                            mybir.AluOpType.mult)
                    hn = L.stage[:, slot, :]
                    nc.vector.tensor_tensor(hn, L.h_prev, d[:, :], add)
                    L.h_prev = L.stage[:, slot, :]
                    if li == 1 and slot == W - 1:
                        nc.sync.dma_start(out_d[:, W * w: W * w + W, :],
                                          L.stage[:, :, :])

                fetch_pre0(0)
                for tt in range(T + LAG):
                    do_gemm = tt >= W and tt % W < GC and (tt // W - 1) < NW
                    if tt < T:
                        emit_step(L0, tt)
                    if do_gemm:
                        emit_gemm_half(tt // W - 1, tt % W, 0)
                    if LAG <= tt < T + LAG:
                        emit_step(L1, tt - LAG)
                    if do_gemm:
                        emit_gemm_half(tt // W - 1, tt % W, 1)

    nc.compile()
    return nc


def _prep_core(inputs, c, shared):
    x = inputs["x"][BL * c: BL * c + BL, :T]          # [4, T, DX]
    xt = np.ascontiguousarray(
        x.transpose(2, 1, 0).reshape(DX, T * BL)).astype(BF16)

    def htr(hv):                                      # [4, H] -> [128, 32]
        o = np.zeros((128, 32), F32)
        for k in range(KC):
            o[:, 4 * k: 4 * k + 4] = hv[:, 128 * k: 128 * k + 128].T
        return o

    h0 = inputs["hx"][0, BL * c: BL * c + BL]
    h1 = inputs["hx"][1, BL * c: BL * c + BL]
    return {
        "xt": xt,
        "h0t": htr(h0).astype(BF16), "h1t": htr(h1).astype(BF16),
        **shared,
    }


def get_nc():
    nc = _CACHE.get("nc")
    if nc is None:
        nc = _build()
        _CACHE["nc"] = nc
    return nc


def make_in_maps(inputs):
    inputs = {k: np.asarray(v) for k, v in inputs.items()}
    shared = {
        "wht0": np.ascontiguousarray(inputs["w_hh_l0"].T).astype(BF16),
        "wht1": np.ascontiguousarray(inputs["w_hh_l1"].T).astype(BF16),
        "wih0t": np.ascontiguousarray(inputs["w_ih_l0"].T).astype(BF16),
        "wih1t": np.ascontiguousarray(inputs["w_ih_l1"].T).astype(BF16),
        "b0c": np.ascontiguousarray(
            (inputs["b_ih_l0"] + inputs["b_hh_l0"]).astype(F32)
            .reshape(GC, 128).T),
        "b1c": np.ascontiguousarray(
            (inputs["b_ih_l1"] + inputs["b_hh_l1"]).astype(F32)
            .reshape(GC, 128).T),
        "idm": np.eye(128, dtype=BF16),
    }
    return [_prep_core(inputs, c, shared) for c in range(NCORES)]


def kernel(**inputs) -> np.ndarray:
    nc = get_nc()
    in_maps = make_in_maps(inputs)
    try:
        res = run_bass_kernel_spmd(nc, in_maps, core_ids=list(range(NCORES)))
    except Exception:
        # a previously wedged device often recovers on the next attempt
        import time
        time.sleep(2.0)
        res = run_bass_kernel_spmd(nc, in_maps, core_ids=list(range(NCORES)))
    outs = []
    for c in range(NCORES):
        o = np.asarray(res.results[c]["out"], F32)      # [128, T, 32]
        o = o.reshape(128, T, KC, BL).transpose(3, 1, 2, 0)
        outs.append(np.ascontiguousarray(o).reshape(BL, T, H))
    return np.concatenate(outs, axis=0).astype(F32)
